# revision 29
# baseline (speedup 1.0000x reference)
"""MHSA block (b=8, c=256, h=w=32, nh=8) on 8 Trainium2 cores.

Sharding: pure data parallel -- one batch element per NeuronCore, no
collectives.  Per-core algorithm (X = x[b] as (C=256, L=1024)):

  QK   = Wqk @ X   (+bqk via a DVE tensor_scalar psum->sbuf copy)     (512, L)
  V^T  = X^T @ Wv^T  (head-padded cols, ones col h*33+32 memset)      (L, 264)
  S^T_h = K_h^T Q_h  (j on partitions; two heads per PE pass via
                      32-row tile_position groups)                    chunks (128, 512)
  P^T  = exp(scale * S^T)            (ScalarE, the ~70us/core floor)
  [O_h; l_h] = V_aug,h^T.T @ P^T_h   (PSUM accumulation over j;
                                      row 32/96 = softmax denominators for free)
  O_norm = O * replicate(1/l)        (reciprocal_approx_fast + K=2 matmul bcast)
  out  = (x + bproj + Wproj @ bv) + Wproj @ O_norm   (biases folded into the
                                                      fp32 residual host-side)

All matmul operands are bf16 (PSUM accumulates fp32).  Schedule notes:
- S^T/exp/PV run as a software pipeline; PV lags 2 iterations so the next
  quad's S^T+exp issue ahead of the previous quad's tail (keeps ScalarE fed
  and the PE dense enough for the HAM clock-gate to grant 2.4 GHz stretches).
- Normalization/projection chains are deferred two quads and placed on the
  "st" PSUM slots mid-quad, so their serial reciprocal path never blocks the
  PE queue head.
- PSUM budget (8 banks): 2x S^T tiles (128,1024) double-buffered = 4, plus
  4x PV accumulators (33,512) = 4.  QK / V^T / norm-chain psum borrows those
  same slots at points where the wait is provably short.
"""

import sys
import os

sys.path.insert(0, "/opt/trn_rl_repo")

from contextlib import ExitStack

import numpy as np

NH, DH, C, L = 8, 32, 256, 1024
B = 8
SCALE = DH ** -0.5
N_CORES = 8


_CACHE = {}


def _build_nc():
    import concourse.tile as tile
    from concourse import bacc, mybir

    f32 = mybir.dt.float32
    bf16 = mybir.dt.bfloat16
    Exp = mybir.ActivationFunctionType.Exp
    Identity = mybir.ActivationFunctionType.Identity

    nc = bacc.Bacc("TRN2", target_bir_lowering=False, debug=False)

    xw_d = nc.dram_tensor("xw", [128, 4624], bf16, kind="ExternalInput").ap()
    xf_d = nc.dram_tensor("xf", [C, L], f32, kind="ExternalInput").ap()
    bqkc_d = nc.dram_tensor("bqkc", [128, 4], f32, kind="ExternalInput").ap()
    e_d = nc.dram_tensor("ee", [2, 128], bf16, kind="ExternalInput").ap()
    out_d = nc.dram_tensor("out", [C, L], f32, kind="ExternalOutput").ap()

    with tile.TileContext(nc) as tc, ExitStack() as ctx:
        persist = ctx.enter_context(tc.tile_pool(name="persist", bufs=1))
        ptpool = ctx.enter_context(tc.tile_pool(name="pt", bufs=6))
        onpool = ctx.enter_context(tc.tile_pool(name="on", bufs=2))
        smallp = ctx.enter_context(tc.tile_pool(name="small", bufs=2))
        stps = ctx.enter_context(tc.tile_pool(name="stps", bufs=2, space="PSUM"))
        pvps = ctx.enter_context(tc.tile_pool(name="pvps", bufs=1, space="PSUM"))

        # ---- x + weights in parallel chunked DMAs (wv/wp deferred) ----
        xw = persist.tile([128, 4624], bf16, tag="xw", name="xw")
        nc.sync.dma_start(xw[:, 0:1024], xw_d[:, 0:1024])
        nc.sync.dma_start(xw[:, 1024:2048], xw_d[:, 1024:2048])
        nc.sync.dma_start(xw[:, 2048:3072], xw_d[:, 2048:3072])
        x_sb = [xw[:, 0:1024], xw[:, 1024:2048]]
        wqk_sb = [xw[:, 2048:2560], xw[:, 2560:3072]]
        wv_sb = [xw[:, 3072:3336], xw[:, 3336:3600]]
        wp_sb = [xw[:, 3600 + 256 * p:3600 + 256 * (p + 1)] for p in range(4)]

        bqkc_sb = persist.tile([128, 4], f32, tag="bqkc", name="bqkc")
        nc.sync.dma_start(bqkc_sb[:], bqkc_d[:])

        # warm the ACT exp table while the QKV phase runs
        warm = persist.tile([1, 8], f32, tag="warm", name="warm")
        nc.gpsimd.memset(warm[:], 0.0)
        nc.scalar.activation(warm[:], warm[:], Exp)

        # ---- QK gemm:  QK(512, L) = WqkT.T @ X; bqk added on the ACT copy ----
        qk_sb = [None] * 4

        def qk_chunk(mt):
            ps = stps.tile([128, L], f32, tag="st", name="qkps")
            for nh_ in range(2):
                o = ps[:, nh_ * 512:(nh_ + 1) * 512]
                for kt in range(2):
                    nc.tensor.matmul(
                        o,
                        lhsT=wqk_sb[kt][:, mt * 128:(mt + 1) * 128],
                        rhs=x_sb[kt][:, nh_ * 512:(nh_ + 1) * 512],
                        start=(kt == 0),
                        stop=(kt == 1),
                    )
            qk = persist.tile([128, L], bf16, tag=f"qk{mt}", name=f"qk{mt}")
            nc.vector.tensor_scalar_add(qk[:], ps[:], bqkc_sb[:, mt:mt + 1])
            qk_sb[mt] = qk

        # ---- V^T gemm: VT(L, 264) = X.T @ WvT + bv  (head-padded cols) ----
        vt_sb = [None] * 8

        def vt_chunk(jt):
            ps = stps.tile([128, 264], f32, tag="st", name="vtps")
            for kt in range(2):
                nc.tensor.matmul(
                    ps[:],
                    lhsT=x_sb[kt][:, jt * 128:(jt + 1) * 128],
                    rhs=wv_sb[kt],
                    start=(kt == 0),
                    stop=(kt == 1),
                )
            vt = persist.tile([128, 264], bf16, tag=f"vt{jt}", name=f"vt{jt}")
            nc.vector.tensor_copy(vt[:], ps[:])
            # softmax-denominator ones column per head
            ones_cols = vt[:].rearrange("p (h c) -> p h c", h=8)[:, :, 32:33]
            nc.gpsimd.memset(ones_cols, 1.0)
            vt_sb[jt] = vt

        qk_chunk(0)
        qk_chunk(2)
        nc.sync.dma_start(xw[:, 3072:4624], xw_d[:, 3072:4624])

        o_sb = []
        for p in range(4):
            o = persist.tile([128, L], f32, tag=f"o{p}", name=f"o{p}")
            nc.gpsimd.memset(o[:], 0.0)
            o_sb.append(o)

        e_sb = persist.tile([2, 128], bf16, tag="ee", name="ee")
        nc.sync.dma_start(e_sb[:], e_d[:])

        acc = [persist.tile([128, L], f32, tag=f"acc{t}", name=f"acc{t}") for t in range(2)]

        xf_sb = []
        for t in range(2):
            xft = persist.tile([128, L], f32, tag=f"xf{t}", name=f"xf{t}")
            nc.sync.dma_start(xft[:], xf_d[t * 128:(t + 1) * 128, :])
            xf_sb.append(xft)

        def make_quad(tg, ih):
            """Returns (st_fns, pv_fns): per-jc emission closures.
            PV emission lags one global iteration behind S^T/exp so the
            next quad's S^T+exp issue ahead of the previous quad's tail."""
            qt = qk_sb[tg]
            kt_ = qk_sb[2 + tg]
            heads = [4 * tg + m for m in range(4)]
            cols = slice(ih * 512, (ih + 1) * 512)
            state = {}

            def st_fn(jc):
                if jc == 4 and deferred:
                    deferred.pop(0)()
                sts = [
                    stps.tile([128, L], f32, tag="st", name="stA"),
                    stps.tile([128, L], f32, tag="st", name="stB"),
                ]
                for m in range(4):
                    o = 32 * m
                    nc.tensor.matmul(
                        sts[m // 2][:, (m % 2) * 512:(m % 2) * 512 + 512],
                        lhsT=kt_[o:o + 32, jc * 128:(jc + 1) * 128],
                        rhs=qt[o:o + 32, cols],
                        start=True,
                        stop=True,
                        tile_position=(o, 0),
                    )
                pts = []
                for half in range(2):
                    pt = ptpool.tile([128, L], bf16, tag="pt", name="pt")
                    nc.scalar.activation(pt[:], sts[half][:], Exp, scale=SCALE)
                    pts.append(pt)
                state[jc] = pts

            def pv_fn(jc):
                if jc == 0:
                    state["pv"] = [
                        pvps.tile([33, 512], f32, tag=f"pv{m}", name=f"pv{m}")
                        for m in range(4)
                    ]
                pts = state.pop(jc)
                pv = state["pv"]
                for m in range(4):
                    h = heads[m]
                    nc.tensor.matmul(
                        pv[m][:],
                        lhsT=vt_sb[jc][:, h * 33:h * 33 + 33],
                        rhs=pts[m // 2][:, (m % 2) * 512:(m % 2) * 512 + 512],
                        start=(jc == 0),
                        stop=(jc == 7),
                    )
                if jc == 7:
                    for m in range(4):
                        pr = 2 * tg + m // 2
                        ro = 64 * (m % 2)
                        nc.vector.tensor_copy(o_sb[pr][ro:ro + 33, cols], pv[m][:])
                    deferred.append(make_norm_chain(tg, cols))

            return st_fn, pv_fn

        def make_norm_chain(tg, cols):
            def norm_chain():
                prs = (2 * tg, 2 * tg + 1)
                ons = []
                rp = stps.tile([128, L], f32, tag="st", name="rp")
                for pi, pr in enumerate(prs):
                    l_sb = smallp.tile([2, 512], f32, tag=f"l{pi}", name="l")
                    nc.sync.dma_start(l_sb[0:1, :], o_sb[pr][32:33, cols])
                    nc.sync.dma_start(l_sb[1:2, :], o_sb[pr][96:97, cols])
                    rl32 = smallp.tile([2, 512], f32, tag=f"rl32{pi}", name="rl32")
                    nc.vector.reciprocal_approx_fast(rl32[:], l_sb[:])
                    rl = smallp.tile([2, 512], bf16, tag=f"rl{pi}", name="rl")
                    nc.vector.tensor_copy(rl[:], rl32[:])

                    rpv = rp[:, pi * 512:(pi + 1) * 512]
                    nc.tensor.matmul(rpv, lhsT=e_sb[:], rhs=rl[:], start=True, stop=True)
                    on = onpool.tile([128, 512], bf16, tag=f"on{pi}", name="on")
                    nc.vector.tensor_mul(on[:], o_sb[pr][:, cols], rpv)
                    ons.append(on)
                pj = stps.tile([128, L], f32, tag="st", name="pj")
                for mt2 in range(2):
                    pjv = pj[:, mt2 * 512:(mt2 + 1) * 512]
                    for pi, pr in enumerate(prs):
                        nc.tensor.matmul(
                            pjv,
                            lhsT=wp_sb[pr][:, mt2 * 128:(mt2 + 1) * 128],
                            rhs=ons[pi][:],
                            start=(pi == 0),
                            stop=(pi == 1),
                        )
                    if tg == 0:
                        nc.vector.tensor_add(acc[mt2][:, cols], xf_sb[mt2][:, cols], pjv)
                    else:
                        nc.vector.tensor_add(acc[mt2][:, cols], acc[mt2][:, cols], pjv)
                        nc.sync.dma_start(out_d[mt2 * 128:(mt2 + 1) * 128, cols], acc[mt2][:, cols])

            return norm_chain

        deferred = []
        for jt in range(8):
            vt_chunk(jt)

        quads = [(0, 0), (0, 1), (1, 0), (1, 1)]
        hooks = {1: [lambda: qk_chunk(1), lambda: qk_chunk(3)]}
        LAG = 2
        pv_queue = []
        for qi, (tg, ih) in enumerate(quads):
            st_fn, pv_fn = make_quad(tg, ih)
            for h in hooks.get(qi, []):
                h()
            for jc in range(8):
                st_fn(jc)
                pv_queue.append((pv_fn, jc))
                if len(pv_queue) > LAG:
                    pj_fn, pjc = pv_queue.pop(0)
                    pj_fn(pjc)
        for pj_fn, pjc in pv_queue:
            pj_fn(pjc)
        while deferred:
            deferred.pop(0)()

    nc.compile()
    return nc


def _get_nc():
    if "nc" not in _CACHE:
        _CACHE["nc"] = _build_nc()
    return _CACHE["nc"]


def _pack_weights(w_qkv, b_qkv, w_proj, b_proj):
    w_qkv = np.asarray(w_qkv, dtype=np.float32)
    b_qkv = np.asarray(b_qkv, dtype=np.float32)
    w_proj = np.asarray(w_proj, dtype=np.float32)
    b_proj = np.asarray(b_proj, dtype=np.float32)

    wqkT = np.ascontiguousarray(w_qkv[:512].T)                  # (256, 512)
    bqkc = np.ascontiguousarray(b_qkv[:512].reshape(4, 128).T)  # (128, 4)

    wvT = np.zeros((C, 264), dtype=np.float32)
    for h in range(NH):
        wvT[:, h * 33:h * 33 + 32] = w_qkv[512 + h * 32:512 + (h + 1) * 32].T

    # o_sb row layout per pair tile p: head 2p at rows 0:32 (l at 32),
    # head 2p+1 at rows 64:96 (l at 96); all other rows zero.
    wpT = np.zeros((512, 256), dtype=np.float32)
    for p in range(4):
        wpT[p * 128 + 0:p * 128 + 32, :] = w_proj[:, (2 * p) * 32:(2 * p + 1) * 32].T
        wpT[p * 128 + 64:p * 128 + 96, :] = w_proj[:, (2 * p + 1) * 32:(2 * p + 2) * 32].T

    ee = np.zeros((2, 128), dtype=np.float32)
    ee[0, 0:32] = 1.0
    ee[1, 64:96] = 1.0
    wblob = np.zeros((128, 2576), dtype=np.float32)
    wblob[:, 0:512] = wqkT[0:128]
    wblob[:, 512:1024] = wqkT[128:256]
    wblob[:, 1024:1288] = wvT[0:128]
    wblob[:, 1288:1552] = wvT[128:256]
    for p in range(4):
        wblob[:, 1552 + 256 * p:1552 + 256 * (p + 1)] = wpT[p * 128:(p + 1) * 128]
    # residual carries x + bproj + Wproj @ bv (the V-bias contribution:
    # O_norm = O/l + bv, and Wproj @ bv is column-constant)
    resid_bias = b_proj + w_proj @ b_qkv[512:768]
    return dict(wblob=wblob, ee=ee), bqkc, resid_bias


def _bf16(a):
    import ml_dtypes

    return np.asarray(a).astype(ml_dtypes.bfloat16)


def _install_ntff_hook_module():
    """bass_utils wants antenv.axon_hooks for trace=True under axon; this
    image's antenv lacks it.  Inject an equivalent module into sys.modules."""
    if "antenv.axon_hooks" in sys.modules:
        return
    try:
        import antenv.axon_hooks  # noqa: F401

        return
    except ImportError:
        pass
    import contextlib
    import ctypes
    import types

    mod = types.ModuleType("antenv.axon_hooks")
    state = {"hook": None, "inited": False}

    def _default_hook():
        so_path = "/opt/axon/libaxon_pjrt.so"
        if not os.path.exists(so_path):
            return None
        lib = ctypes.CDLL(so_path)
        if not hasattr(lib, "axon_start_nrt_profile"):
            return None
        lib.axon_start_nrt_profile.argtypes = [
            ctypes.POINTER(ctypes.c_int64),
            ctypes.c_size_t,
        ]
        lib.axon_start_nrt_profile.restype = ctypes.c_int64
        lib.axon_stop_nrt_profile.argtypes = [ctypes.c_char_p]
        lib.axon_stop_nrt_profile.restype = ctypes.c_int64

        @contextlib.contextmanager
        def _hook(output_dir, device_ids):
            import jax

            jax.devices()
            if device_ids:
                ids = (ctypes.c_int64 * len(device_ids))(*device_ids)
                rc = lib.axon_start_nrt_profile(ids, len(device_ids))
            else:
                rc = lib.axon_start_nrt_profile(None, 0)
            if rc != 0:
                raise RuntimeError(f"axon_start_nrt_profile rc={rc}")
            try:
                yield
            finally:
                n = lib.axon_stop_nrt_profile(str(output_dir).encode())
                if n < 0:
                    raise RuntimeError(f"axon_stop_nrt_profile rc={n}")
                print(f"profile: {n} file(s) written to {output_dir}")

        return _hook

    def set_axon_ntff_profile_hook(hook):
        state["hook"] = hook
        state["inited"] = True

    def get_axon_ntff_profile_hook():
        if not state["inited"]:
            state["hook"] = _default_hook()
            state["inited"] = True
        return state["hook"]

    mod.set_axon_ntff_profile_hook = set_axon_ntff_profile_hook
    mod.get_axon_ntff_profile_hook = get_axon_ntff_profile_hook
    sys.modules["antenv.axon_hooks"] = mod


def _prepare_in_maps(x, w_qkv, b_qkv, w_proj, b_proj):
    x = np.asarray(x, dtype=np.float32)
    b, c, h, w = x.shape
    assert (b, c, h, w) == (B, C, 32, 32)

    weights, bqkc, resid_bias = _pack_weights(w_qkv, b_qkv, w_proj, b_proj)
    weights = {k: _bf16(v) for k, v in weights.items()}
    weights["bqkc"] = np.ascontiguousarray(bqkc, dtype=np.float32)

    wblob_bf = weights.pop("wblob")
    in_maps = []
    for core in range(N_CORES):
        m = dict(weights)
        xm = np.ascontiguousarray(x[core].reshape(C, L))
        xw = np.empty((128, 4624), dtype=wblob_bf.dtype)
        xw[:, 0:1024] = _bf16(xm[0:128])
        xw[:, 1024:2048] = _bf16(xm[128:256])
        xw[:, 2048:4624] = wblob_bf
        m["xw"] = xw
        m["xf"] = xm + resid_bias[:, None].astype(np.float32)
        in_maps.append(m)
    return in_maps


def kernel(x, w_qkv, b_qkv, w_proj, b_proj, _trace=False, _trace_kwargs=None):
    if _trace:
        _install_ntff_hook_module()
    from concourse.bass_utils import run_bass_kernel_spmd

    in_maps = _prepare_in_maps(x, w_qkv, b_qkv, w_proj, b_proj)
    nc = _get_nc()

    res = run_bass_kernel_spmd(
        nc,
        in_maps,
        list(range(N_CORES)),
        trace=_trace,
        **(_trace_kwargs or {}),
    )
    out = np.stack([res.results[core]["out"] for core in range(N_CORES)])
    if _trace:
        _CACHE["last_result"] = res
    return out.reshape(B, C, 32, 32)


# revision 30
# speedup vs baseline: 1.1035x; 1.1035x over previous
"""MHSA block (b=8, c=256, h=w=32, nh=8) on 8 Trainium2 cores.

Sharding: pure data parallel -- one batch element per NeuronCore, no
collectives.  Per-core algorithm (X = x[b] as (C=256, L=1024)):

  QK   = Wqk @ X   (+bqk via a DVE tensor_scalar psum->sbuf copy)     (512, L)
  V^T  = X^T @ Wv^T  (head-padded cols, ones col h*33+32 memset)      (L, 264)
  S^T_h = K_h^T Q_h  (j on partitions; two heads per PE pass via
                      32-row tile_position groups)                    chunks (128, 512)
  P^T  = exp(scale * S^T)            (ScalarE, the ~70us/core floor)
  [O_h; l_h] = V_aug,h^T.T @ P^T_h   (PSUM accumulation over j;
                                      row 32/96 = softmax denominators for free)
  O_norm = O * replicate(1/l)        (reciprocal_approx_fast + K=2 matmul bcast)
  out  = (x + bproj + Wproj @ bv) + Wproj @ O_norm   (biases folded into the
                                                      fp32 residual host-side)

All matmul operands are bf16 (PSUM accumulates fp32).  Schedule notes:
- S^T/exp/PV run as a software pipeline; PV lags 2 iterations so the next
  quad's S^T+exp issue ahead of the previous quad's tail (keeps ScalarE fed
  and the PE dense enough for the HAM clock-gate to grant 2.4 GHz stretches).
- Normalization/projection chains are deferred two quads and placed on the
  "st" PSUM slots mid-quad, so their serial reciprocal path never blocks the
  PE queue head.
- PSUM budget (8 banks): 2x S^T tiles (128,1024) double-buffered = 4, plus
  4x PV accumulators (33,512) = 4.  QK / V^T / norm-chain psum borrows those
  same slots at points where the wait is provably short.
"""

import sys
import os

sys.path.insert(0, "/opt/trn_rl_repo")

from contextlib import ExitStack

import numpy as np

NH, DH, C, L = 8, 32, 256, 1024
B = 8
SCALE = DH ** -0.5
N_CORES = 8


_CACHE = {}


def _build_nc():
    import concourse.tile as tile
    from concourse import bacc, mybir

    f32 = mybir.dt.float32
    bf16 = mybir.dt.bfloat16
    Exp = mybir.ActivationFunctionType.Exp
    Identity = mybir.ActivationFunctionType.Identity

    nc = bacc.Bacc("TRN2", target_bir_lowering=False, debug=False)

    xw_d = nc.dram_tensor("xw", [128, 4624], bf16, kind="ExternalInput").ap()
    xf_d = nc.dram_tensor("xf", [C, L], f32, kind="ExternalInput").ap()
    bqkc_d = nc.dram_tensor("bqkc", [128, 4], f32, kind="ExternalInput").ap()
    e_d = nc.dram_tensor("ee", [2, 128], bf16, kind="ExternalInput").ap()
    out_d = nc.dram_tensor("out", [C, L], f32, kind="ExternalOutput").ap()

    with tile.TileContext(nc) as tc, ExitStack() as ctx:
        persist = ctx.enter_context(tc.tile_pool(name="persist", bufs=1))
        ptpool = ctx.enter_context(tc.tile_pool(name="pt", bufs=6))
        onpool = ctx.enter_context(tc.tile_pool(name="on", bufs=2))
        smallp = ctx.enter_context(tc.tile_pool(name="small", bufs=2))
        stps = ctx.enter_context(tc.tile_pool(name="stps", bufs=2, space="PSUM"))
        pvps = ctx.enter_context(tc.tile_pool(name="pvps", bufs=1, space="PSUM"))

        # ---- x + weights in parallel chunked DMAs (wv/wp deferred) ----
        xw = persist.tile([128, 4624], bf16, tag="xw", name="xw")
        nc.sync.dma_start(xw[:, 0:1024], xw_d[:, 0:1024])
        nc.sync.dma_start(xw[:, 1024:2048], xw_d[:, 1024:2048])
        nc.sync.dma_start(xw[:, 2048:3072], xw_d[:, 2048:3072])
        x_sb = [xw[:, 0:1024], xw[:, 1024:2048]]
        wqk_sb = [xw[:, 2048:2560], xw[:, 2560:3072]]
        wv_sb = [xw[:, 3072:3336], xw[:, 3336:3600]]
        wp_sb = [xw[:, 3600 + 256 * p:3600 + 256 * (p + 1)] for p in range(4)]

        bqkc_sb = persist.tile([128, 4], f32, tag="bqkc", name="bqkc")
        nc.sync.dma_start(bqkc_sb[:], bqkc_d[:])

        # warm the ACT exp table while the QKV phase runs
        warm = persist.tile([1, 8], f32, tag="warm", name="warm")
        nc.gpsimd.memset(warm[:], 0.0)
        nc.scalar.activation(warm[:], warm[:], Exp)

        # ---- QK gemm:  QK(512, L) = WqkT.T @ X; bqk added on the ACT copy ----
        qk_sb = [None] * 4

        def qk_chunk(mt):
            ps = stps.tile([128, L], f32, tag="st", name="qkps")
            for nh_ in range(2):
                o = ps[:, nh_ * 512:(nh_ + 1) * 512]
                for kt in range(2):
                    nc.tensor.matmul(
                        o,
                        lhsT=wqk_sb[kt][:, mt * 128:(mt + 1) * 128],
                        rhs=x_sb[kt][:, nh_ * 512:(nh_ + 1) * 512],
                        start=(kt == 0),
                        stop=(kt == 1),
                    )
            qk = persist.tile([128, L], bf16, tag=f"qk{mt}", name=f"qk{mt}")
            nc.vector.tensor_scalar_add(qk[:], ps[:], bqkc_sb[:, mt:mt + 1])
            qk_sb[mt] = qk

        # ---- V^T gemm: VT(L, 264) = X.T @ WvT + bv  (head-padded cols) ----
        vt_sb = [None] * 8

        def vt_chunk(jt):
            ps = stps.tile([128, 264], f32, tag="st", name="vtps")
            for kt in range(2):
                nc.tensor.matmul(
                    ps[:],
                    lhsT=x_sb[kt][:, jt * 128:(jt + 1) * 128],
                    rhs=wv_sb[kt],
                    start=(kt == 0),
                    stop=(kt == 1),
                )
            vt = persist.tile([128, 264], bf16, tag=f"vt{jt}", name=f"vt{jt}")
            nc.vector.tensor_copy(vt[:], ps[:])
            # softmax-denominator ones column per head
            ones_cols = vt[:].rearrange("p (h c) -> p h c", h=8)[:, :, 32:33]
            nc.gpsimd.memset(ones_cols, 1.0)
            vt_sb[jt] = vt

        qk_chunk(0)
        qk_chunk(2)
        nc.sync.dma_start(xw[:, 3072:4624], xw_d[:, 3072:4624])

        o_sb = []
        for p in range(4):
            o = persist.tile([128, L], f32, tag=f"o{p}", name=f"o{p}")
            nc.gpsimd.memset(o[:], 0.0)
            o_sb.append(o)

        e_sb = persist.tile([2, 128], bf16, tag="ee", name="ee")
        nc.sync.dma_start(e_sb[:], e_d[:])

        acc = [persist.tile([128, L], f32, tag=f"acc{t}", name=f"acc{t}") for t in range(2)]

        xf_sb = []
        for t in range(2):
            xft = persist.tile([128, L], f32, tag=f"xf{t}", name=f"xf{t}")
            nc.sync.dma_start(xft[:], xf_d[t * 128:(t + 1) * 128, :])
            xf_sb.append(xft)

        def make_quad(tg, ih):
            """Returns (st_fns, pv_fns): per-jc emission closures.
            PV emission lags one global iteration behind S^T/exp so the
            next quad's S^T+exp issue ahead of the previous quad's tail."""
            qt = qk_sb[tg]
            kt_ = qk_sb[2 + tg]
            heads = [4 * tg + m for m in range(4)]
            cols = slice(ih * 512, (ih + 1) * 512)
            state = {}

            def st_fn(jc):
                if jc == 4 and len(deferred) >= 2:
                    deferred.pop(0)()
                sts = [
                    stps.tile([128, L], f32, tag="st", name="stA"),
                    stps.tile([128, L], f32, tag="st", name="stB"),
                ]
                for m in range(4):
                    o = 32 * m
                    nc.tensor.matmul(
                        sts[m // 2][:, (m % 2) * 512:(m % 2) * 512 + 512],
                        lhsT=kt_[o:o + 32, jc * 128:(jc + 1) * 128],
                        rhs=qt[o:o + 32, cols],
                        start=True,
                        stop=True,
                        tile_position=(o, 0),
                    )
                pts = []
                for half in range(2):
                    pt = ptpool.tile([128, L], bf16, tag="pt", name="pt")
                    nc.scalar.activation(pt[:], sts[half][:], Exp, scale=SCALE)
                    pts.append(pt)
                state[jc] = pts

            def pv_fn(jc):
                if jc == 0:
                    state["pv"] = [
                        pvps.tile([33, 512], f32, tag=f"pv{m}", name=f"pv{m}")
                        for m in range(4)
                    ]
                pts = state.pop(jc)
                pv = state["pv"]
                for m in range(4):
                    h = heads[m]
                    nc.tensor.matmul(
                        pv[m][:],
                        lhsT=vt_sb[jc][:, h * 33:h * 33 + 33],
                        rhs=pts[m // 2][:, (m % 2) * 512:(m % 2) * 512 + 512],
                        start=(jc == 0),
                        stop=(jc == 7),
                    )
                if jc == 7:
                    for m in range(4):
                        pr = 2 * tg + m // 2
                        ro = 64 * (m % 2)
                        nc.vector.tensor_copy(o_sb[pr][ro:ro + 33, cols], pv[m][:])
                    deferred.append(make_norm_chain(tg, cols))

            return st_fn, pv_fn

        def make_norm_chain(tg, cols):
            def norm_chain():
                prs = (2 * tg, 2 * tg + 1)
                ons = []
                rp = stps.tile([128, L], f32, tag="st", name="rp")
                for pi, pr in enumerate(prs):
                    l_sb = smallp.tile([2, 512], f32, tag=f"l{pi}", name="l")
                    nc.sync.dma_start(l_sb[0:1, :], o_sb[pr][32:33, cols])
                    nc.sync.dma_start(l_sb[1:2, :], o_sb[pr][96:97, cols])
                    rl32 = smallp.tile([2, 512], f32, tag=f"rl32{pi}", name="rl32")
                    nc.vector.reciprocal_approx_fast(rl32[:], l_sb[:])
                    rl = smallp.tile([2, 512], bf16, tag=f"rl{pi}", name="rl")
                    nc.vector.tensor_copy(rl[:], rl32[:])

                    rpv = rp[:, pi * 512:(pi + 1) * 512]
                    nc.tensor.matmul(rpv, lhsT=e_sb[:], rhs=rl[:], start=True, stop=True)
                    on = onpool.tile([128, 512], bf16, tag=f"on{pi}", name="on")
                    nc.vector.tensor_mul(on[:], o_sb[pr][:, cols], rpv)
                    ons.append(on)
                pj = stps.tile([128, L], f32, tag="st", name="pj")
                for mt2 in range(2):
                    pjv = pj[:, mt2 * 512:(mt2 + 1) * 512]
                    for pi, pr in enumerate(prs):
                        nc.tensor.matmul(
                            pjv,
                            lhsT=wp_sb[pr][:, mt2 * 128:(mt2 + 1) * 128],
                            rhs=ons[pi][:],
                            start=(pi == 0),
                            stop=(pi == 1),
                        )
                    if tg == 0:
                        nc.vector.tensor_add(acc[mt2][:, cols], xf_sb[mt2][:, cols], pjv)
                    else:
                        nc.vector.tensor_add(acc[mt2][:, cols], acc[mt2][:, cols], pjv)
                        nc.sync.dma_start(out_d[mt2 * 128:(mt2 + 1) * 128, cols], acc[mt2][:, cols])

            return norm_chain

        deferred = []
        for jt in range(8):
            vt_chunk(jt)

        quads = [(0, 0), (0, 1), (1, 0), (1, 1)]
        hooks = {1: [lambda: qk_chunk(1), lambda: qk_chunk(3)]}
        LAG = 2
        pv_queue = []
        for qi, (tg, ih) in enumerate(quads):
            st_fn, pv_fn = make_quad(tg, ih)
            for h in hooks.get(qi, []):
                h()
            for jc in range(8):
                st_fn(jc)
                pv_queue.append((pv_fn, jc))
                if len(pv_queue) > LAG:
                    pj_fn, pjc = pv_queue.pop(0)
                    pj_fn(pjc)
        for pj_fn, pjc in pv_queue:
            pj_fn(pjc)
        while deferred:
            deferred.pop(0)()

    nc.compile()
    return nc


def _get_nc():
    if "nc" not in _CACHE:
        _CACHE["nc"] = _build_nc()
    return _CACHE["nc"]


def _pack_weights(w_qkv, b_qkv, w_proj, b_proj):
    w_qkv = np.asarray(w_qkv, dtype=np.float32)
    b_qkv = np.asarray(b_qkv, dtype=np.float32)
    w_proj = np.asarray(w_proj, dtype=np.float32)
    b_proj = np.asarray(b_proj, dtype=np.float32)

    wqkT = np.ascontiguousarray(w_qkv[:512].T)                  # (256, 512)
    bqkc = np.ascontiguousarray(b_qkv[:512].reshape(4, 128).T)  # (128, 4)

    wvT = np.zeros((C, 264), dtype=np.float32)
    for h in range(NH):
        wvT[:, h * 33:h * 33 + 32] = w_qkv[512 + h * 32:512 + (h + 1) * 32].T

    # o_sb row layout per pair tile p: head 2p at rows 0:32 (l at 32),
    # head 2p+1 at rows 64:96 (l at 96); all other rows zero.
    wpT = np.zeros((512, 256), dtype=np.float32)
    for p in range(4):
        wpT[p * 128 + 0:p * 128 + 32, :] = w_proj[:, (2 * p) * 32:(2 * p + 1) * 32].T
        wpT[p * 128 + 64:p * 128 + 96, :] = w_proj[:, (2 * p + 1) * 32:(2 * p + 2) * 32].T

    ee = np.zeros((2, 128), dtype=np.float32)
    ee[0, 0:32] = 1.0
    ee[1, 64:96] = 1.0
    wblob = np.zeros((128, 2576), dtype=np.float32)
    wblob[:, 0:512] = wqkT[0:128]
    wblob[:, 512:1024] = wqkT[128:256]
    wblob[:, 1024:1288] = wvT[0:128]
    wblob[:, 1288:1552] = wvT[128:256]
    for p in range(4):
        wblob[:, 1552 + 256 * p:1552 + 256 * (p + 1)] = wpT[p * 128:(p + 1) * 128]
    # residual carries x + bproj + Wproj @ bv (the V-bias contribution:
    # O_norm = O/l + bv, and Wproj @ bv is column-constant)
    resid_bias = b_proj + w_proj @ b_qkv[512:768]
    return dict(wblob=wblob, ee=ee), bqkc, resid_bias


def _bf16(a):
    import ml_dtypes

    return np.asarray(a).astype(ml_dtypes.bfloat16)


def _install_ntff_hook_module():
    """bass_utils wants antenv.axon_hooks for trace=True under axon; this
    image's antenv lacks it.  Inject an equivalent module into sys.modules."""
    if "antenv.axon_hooks" in sys.modules:
        return
    try:
        import antenv.axon_hooks  # noqa: F401

        return
    except ImportError:
        pass
    import contextlib
    import ctypes
    import types

    mod = types.ModuleType("antenv.axon_hooks")
    state = {"hook": None, "inited": False}

    def _default_hook():
        so_path = "/opt/axon/libaxon_pjrt.so"
        if not os.path.exists(so_path):
            return None
        lib = ctypes.CDLL(so_path)
        if not hasattr(lib, "axon_start_nrt_profile"):
            return None
        lib.axon_start_nrt_profile.argtypes = [
            ctypes.POINTER(ctypes.c_int64),
            ctypes.c_size_t,
        ]
        lib.axon_start_nrt_profile.restype = ctypes.c_int64
        lib.axon_stop_nrt_profile.argtypes = [ctypes.c_char_p]
        lib.axon_stop_nrt_profile.restype = ctypes.c_int64

        @contextlib.contextmanager
        def _hook(output_dir, device_ids):
            import jax

            jax.devices()
            if device_ids:
                ids = (ctypes.c_int64 * len(device_ids))(*device_ids)
                rc = lib.axon_start_nrt_profile(ids, len(device_ids))
            else:
                rc = lib.axon_start_nrt_profile(None, 0)
            if rc != 0:
                raise RuntimeError(f"axon_start_nrt_profile rc={rc}")
            try:
                yield
            finally:
                n = lib.axon_stop_nrt_profile(str(output_dir).encode())
                if n < 0:
                    raise RuntimeError(f"axon_stop_nrt_profile rc={n}")
                print(f"profile: {n} file(s) written to {output_dir}")

        return _hook

    def set_axon_ntff_profile_hook(hook):
        state["hook"] = hook
        state["inited"] = True

    def get_axon_ntff_profile_hook():
        if not state["inited"]:
            state["hook"] = _default_hook()
            state["inited"] = True
        return state["hook"]

    mod.set_axon_ntff_profile_hook = set_axon_ntff_profile_hook
    mod.get_axon_ntff_profile_hook = get_axon_ntff_profile_hook
    sys.modules["antenv.axon_hooks"] = mod


def _prepare_in_maps(x, w_qkv, b_qkv, w_proj, b_proj):
    x = np.asarray(x, dtype=np.float32)
    b, c, h, w = x.shape
    assert (b, c, h, w) == (B, C, 32, 32)

    weights, bqkc, resid_bias = _pack_weights(w_qkv, b_qkv, w_proj, b_proj)
    weights = {k: _bf16(v) for k, v in weights.items()}
    weights["bqkc"] = np.ascontiguousarray(bqkc, dtype=np.float32)

    wblob_bf = weights.pop("wblob")
    in_maps = []
    for core in range(N_CORES):
        m = dict(weights)
        xm = np.ascontiguousarray(x[core].reshape(C, L))
        xw = np.empty((128, 4624), dtype=wblob_bf.dtype)
        xw[:, 0:1024] = _bf16(xm[0:128])
        xw[:, 1024:2048] = _bf16(xm[128:256])
        xw[:, 2048:4624] = wblob_bf
        m["xw"] = xw
        m["xf"] = xm + resid_bias[:, None].astype(np.float32)
        in_maps.append(m)
    return in_maps


def kernel(x, w_qkv, b_qkv, w_proj, b_proj, _trace=False, _trace_kwargs=None):
    if _trace:
        _install_ntff_hook_module()
    from concourse.bass_utils import run_bass_kernel_spmd

    in_maps = _prepare_in_maps(x, w_qkv, b_qkv, w_proj, b_proj)
    nc = _get_nc()

    res = run_bass_kernel_spmd(
        nc,
        in_maps,
        list(range(N_CORES)),
        trace=_trace,
        **(_trace_kwargs or {}),
    )
    out = np.stack([res.results[core]["out"] for core in range(N_CORES)])
    if _trace:
        _CACHE["last_result"] = res
    return out.reshape(B, C, 32, 32)
